# revision 25
# baseline (speedup 1.0000x reference)
"""Trainium2 Bass kernel for BinaryTreeLatentVariable inside algorithm.

Math (per level, bottom-up over a complete binary tree in heap order):
    new[pp, n] = p[pp, n] + logsumexp_{i,j}( trans[pp,i,j] + l[i,n] + r[j,n] )

CP factorization: exp(trans)[pp,i,j] ~= sum_r U[pp,r] V[i,r] W[j,r] (rank-32
ALS fit):
    S[pp, n] = sum_r U[pp,r] * (V^T Fl)[r,n] * (W^T Fr)[r,n]
with F the child values in EXP space.

Level scheme: FULL (ln renormalize, capture per-column z) at levels {6, 3}
only; every other internal level is FAST (pure exp space) with a per-level
power-of-two scale 2^-c_d folded into that level's U matrix to bound the
magnitude drift (values stay within e^+-45; bf16 max e^88).  Host repays
sum(c_d * 2^d) * ln2 + 72*b[0] per tree as a constant.

Per FAST level: mm V^T Fl and W^T Fr into adjacent psum columns (same
partitions 0..31) -> one DVE mult psum*psum -> bf16 vb -> mm U ->
DVE mult by exp(sw_raw) into the level buffer.  No cast, no memsets.
Per FULL level: ... -> mm U -> ACT ln -> psum (normmat@t + I@sw_norm) via
two accumulating matmuls -> ACT exp straight from psum into the level
buffer; t row 0 reduced per tree on GpSimd for z.

Left/right child values live in separate tiles (yL/yR), both at partitions
0..19, parent-column indexed (bit-reversed node order, 8 trees innermost).

Emission: h cast host-side to fp8e4, two DoubleRow matmuls (K=256 each) per
512-col tile into [64,1024] psum super-tiles (two tiles per ACT where the
destination allows), producing a raw block (rows 32..51, consumed via Exp)
and a normalized block (rows 0..19, Identity, for FULL levels) per column.

Column order puts the first leaf L/R pair and the whole L8 block early so
tree levels start while h still streams; 12 junk matmuls at t=0 trip the
PE HAM clock gate to 2.4 GHz before the first real matmul.

Sharding: 8 trees per core across 8 cores (no cross-core communication).
"""

import ml_dtypes
import numpy as np

import concourse.bacc as bacc
import concourse.bass as bass
from concourse import mybir, tile
from concourse.bass_utils import run_bass_kernel_spmd

F32 = mybir.dt.float32
BF16 = mybir.dt.bfloat16
FP8 = mybir.dt.float8e4
NP_BF16 = ml_dtypes.bfloat16
NP_FP8 = ml_dtypes.float8_e4m3

B = 64
N_NODES = 1023
D = 512
L = 5
C = 4
LC = L * C          # 20
NCORES = 8
TPC = B // NCORES   # trees per core = 8
DEPTH = 9           # leaves are level 9; internal levels 8..0
R = 32              # CP rank
LN2 = float(np.log(2.0))

FULL = {6, 3}
# per-level scale exponent: U_d = U * 2^-C_D[d] (host-repaid).  FAST levels
# manage bf16 range; FULL/root values CENTER the ln input near e^0..e^14 --
# the ACT spline is inaccurate far outside that (the scale cancels in t-t0,
# so FULL-level c only moves the ln input and the z constant).
C_D = {8: 20, 7: 16, 6: -50, 5: 20, 4: 16, 3: -52, 2: 20, 1: 12, 0: -43}
ZCON_SHIFT = LN2 * sum(C_D[d] * (1 << d) for d in C_D)  # 10204*ln2
N_FULL_NODES = 64 + 8   # level-6 + level-3 nodes per tree

# sw_sb internal-level offsets (level-major, L8 first)
OFF = {}
_o = 0
for _d in range(8, -1, -1):
    OFF[_d] = _o
    _o += TPC * (1 << _d)
NCOLI = 4096            # internal cols incl 8 pad

# global emission column order (16 tiles of 512):
#  t0 leafL0 | t1 leafR0 | t2-5 L8 | t6 leafL1 | t7 leafR1 | t8 leafL2
#  t9 leafR2 | t10 leafL3 | t11 leafR3 | t12-13 L7 | t14 L6 | t15 rest
CHUNK_COLS = [1024, 2048, 2048, 2048, 1024]   # 5 h DMA chunks
CHUNK_OFF = [0, 1024, 3072, 5120, 7168]


def _cp_fit(trans):
    """Rank-R ALS CP fit of exp(trans) as [pp,(lL,lc),(rL,rc)]."""
    T = np.exp(trans.astype(np.float64).transpose(0, 3, 1, 4, 2, 5)
               .reshape(LC, LC, LC))
    rng = np.random.default_rng(0)
    U = rng.uniform(0.5, 1.5, (LC, R))
    V = rng.uniform(0.5, 1.5, (LC, R))
    W = rng.uniform(0.5, 1.5, (LC, R))
    T1 = T.reshape(LC, -1)
    T2 = T.transpose(1, 0, 2).reshape(LC, -1)
    T3 = T.transpose(2, 0, 1).reshape(LC, -1)

    def khatri(A, Bm):
        return (A[:, None, :] * Bm[None, :, :]).reshape(-1, A.shape[1])

    eye = 1e-10 * np.eye(R)
    for _ in range(200):
        for mode in range(3):
            if mode == 0:
                K, M = khatri(V, W), T1
            elif mode == 1:
                K, M = khatri(U, W), T2
            else:
                K, M = khatri(U, V), T3
            X = np.linalg.solve(K.T @ K + eye, (M @ K).T).T
            if mode == 0:
                U = X
            elif mode == 1:
                V = X
            else:
                W = X
    sv = np.abs(V).max(0)
    sw = np.abs(W).max(0)
    return U * (sv * sw), V / sv, W / sw


def _host_constants(W, b, trans):
    W = W.astype(np.float64)
    b = b.astype(np.float64)
    U, Vf, Wf = _cp_fit(trans)

    # emission weights, 64 columns: 0..19 normalized (col i = W_i - W_0,
    # col 0 = W_0 whose psum row is the sw0 z-capture), 32..51 raw
    Wn = np.zeros((D, 64))
    Wn[:, 0] = W[:, 0]
    Wn[:, 1:LC] = W[:, 1:] - W[:, 0:1]
    Wn[:, 32:32 + LC] = W
    esc = float(2.0 ** np.floor(np.log2(235.0 / np.abs(Wn).max())))
    wq = np.clip(Wn * esc, -240, 240).astype(NP_FP8)
    # [p, P, ko, m]: row (P*256 + ko*128 + p) -> w5[p, P, ko, m]
    w5 = np.ascontiguousarray(
        wq.reshape(2, 2, 128, 64).transpose(2, 0, 1, 3))

    # bf16 pack [52, 224]: Vf | Wf | u variants | normmat | idmat
    bpack = np.zeros((52, 224), NP_BF16)
    bpack[0:LC, 0:R] = Vf
    bpack[0:LC, R:2 * R] = Wf
    for j, c in enumerate(sorted(set(C_D.values()))):
        bpack[0:R, 64 + j * LC:64 + (j + 1) * LC] = (U * 2.0 ** (-c)).T
    nm = np.zeros((LC, LC))
    for i in range(1, LC):
        nm[i, i] = 1.0
        nm[0, i] = -1.0
    bpack[0:LC, 184:204] = nm
    bpack[0:LC, 204:224] = np.eye(LC)

    # f32 pack [20, 40]: col0 escn col1 ebin col2 escr col3 ebir;
    # row0: onesr [4:24], zcon [24:32], zscale [32:40]
    fpack = np.zeros((LC, 40), np.float32)
    fpack[1:, 0] = 1.0 / esc
    fpack[1:, 1] = b[1:] - b[0]
    fpack[:, 2] = 1.0 / esc
    fpack[:, 3] = b
    fpack[0, 4:24] = 1.0
    fpack[0, 24:32] = ZCON_SHIFT + N_FULL_NODES * b[0]
    fpack[0, 32:40] = 1.0 / esc
    return {"w5": w5, "bpack": bpack, "fpack": fpack}


def _bitrev(d):
    n = 1 << d
    perm = np.zeros(n, np.int64)
    for x in range(n):
        v, q = x, 0
        for _ in range(d):
            q = (q << 1) | (v & 1)
            v >>= 1
        perm[x] = q
    return perm


def _host_ht(h, core):
    """fp8 chunk list for one core in the new column order."""
    hk = h[core * TPC:(core + 1) * TPC]          # [8, 1023, 512]

    def lvl(d):
        blk = hk[:, (1 << d) - 1:(1 << (d + 1)) - 1, :]
        blk = blk[:, _bitrev(d), :]
        return blk.transpose(2, 1, 0).reshape(D, -1)   # col = x*8+t

    l9 = lvl(9)                 # [512, 4096]
    l9L, l9R = l9[:, :2048], l9[:, 2048:]
    blocks = [l9L[:, 0:512], l9R[:, 0:512], lvl(8),
              l9L[:, 512:1024], l9R[:, 512:1024],
              l9L[:, 1024:1536], l9R[:, 1024:1536],
              l9L[:, 1536:2048], l9R[:, 1536:2048],
              lvl(7), lvl(6), lvl(5), lvl(4), lvl(3), lvl(2), lvl(1),
              lvl(0), np.zeros((D, 8), np.float32)]
    out = np.concatenate(blocks, axis=1)          # [512, 8192]
    hq = np.clip(out, -240, 240).astype(NP_FP8)
    # row (P*256 + ko*128 + p) -> [p, P, ko, col]
    h4 = hq.reshape(2, 2, 128, 8192).transpose(2, 0, 1, 3)
    return [np.ascontiguousarray(h4[:, :, :, o:o + n])
            for o, n in zip(CHUNK_OFF, CHUNK_COLS)]


def _patch_act_tables(nc):
    """Retarget every activation-table load to natural_log_exp_and_others
    (covers Exp, Ln and Identity) and drop the now-redundant reloads."""
    from concourse.hw_specs import get_activation_tables
    tables = list(get_activation_tables(nc.m.arch).items())
    target = None
    for idx, (name, _fns) in enumerate(tables):
        if name == "natural_log_exp_and_others":
            target = idx
    if target is None:
        return
    for fn in nc.m.functions:
        kept = False
        for blk in fn.blocks:
            new_insts = []
            for ins in blk.instructions:
                if isinstance(ins, mybir.InstLoadActFuncSet):
                    si = ins.sync_info
                    has_sems = si is not None and (
                        len(si.on_wait) > 0 or len(si.on_update) > 0)
                    if not kept or has_sems:
                        ins.act_func_set_id = target
                        kept = True
                        new_insts.append(ins)
                    continue
                new_insts.append(ins)
            blk.instructions[:] = new_insts


def _build_bass():
    nc = bacc.Bacc("TRN2", target_bir_lowering=False)

    ht_d = [nc.declare_dram_parameter(f"ht{i}", [128, 2, 2, n], FP8,
                                      isOutput=False)
            for i, n in enumerate(CHUNK_COLS)]
    w5_d = nc.declare_dram_parameter("w5", [128, 2, 2, 64], FP8,
                                     isOutput=False)
    bpack_d = nc.declare_dram_parameter("bpack", [52, 224], BF16,
                                        isOutput=False)
    fpack_d = nc.declare_dram_parameter("fpack", [LC, 40], F32,
                                        isOutput=False)
    out_d = nc.declare_dram_parameter("out", [LC, TPC], F32, isOutput=True)

    EXP = mybir.ActivationFunctionType.Exp
    LN = mybir.ActivationFunctionType.Ln
    IDENT = mybir.ActivationFunctionType.Identity
    ADD = mybir.AluOpType.add
    MULT = mybir.AluOpType.mult
    DR = mybir.MatmulPerfMode.DoubleRow
    AXX = mybir.AxisListType.X

    USLOT = {c: j for j, c in enumerate(sorted(set(C_D.values())))}
    # zparts slots: 0 t0_L6, 1 t0_L3, 2 sw0_L6, 3 sw0_L3
    with tile.TileContext(nc) as tc:
        with (
            tc.tile_pool(name="consts", bufs=1) as consts,
            tc.tile_pool(name="sw", bufs=1) as swp,
            tc.tile_pool(name="ybufs", bufs=1) as ybp,
            tc.tile_pool(name="ht", bufs=2) as htp,
            tc.tile_pool(name="vtiles", bufs=2) as vtp,
            tc.tile_pool(name="ttiles", bufs=2) as ttp,
            tc.tile_pool(name="ps_em", bufs=2, space="PSUM") as ps_emp,
            tc.tile_pool(name="ps_a", bufs=2, space="PSUM") as ps_ap,
        ):
            # ---- DMAs first: h chunks on sync queue, consts on scalar ----
            htts = []
            for i, ncol in enumerate(CHUNK_COLS):
                t = htp.tile([128, 2, 2, ncol], FP8, tag=f"ht{i % 2}",
                             name=f"ht{i}")
                nc.sync.dma_start(t[:], ht_d[i][:])
                htts.append(t)
            # junk tiles for PE warm-up matmuls (memsets first so the
            # junk matmuls start immediately)
            junk_w = consts.tile([128, 2, 64], FP8)
            junk_h = consts.tile([128, 2, 256], FP8)
            nc.gpsimd.memset(junk_w[:], 0.0)
            nc.gpsimd.memset(junk_h[:], 0.0)
            # consts via gpsimd SWDGE: keeps the scalar/ACT queue free so
            # the activation-table load runs at t=0
            w_sb = consts.tile([128, 2, 2, 64], FP8)
            nc.gpsimd.dma_start(w_sb[:], w5_d[:])
            bp = consts.tile([52, 224], BF16)
            nc.gpsimd.dma_start(bp[:], bpack_d[:])
            fp = consts.tile([LC, 40], F32)
            nc.gpsimd.dma_start(fp[:], fpack_d[:])

            vf_ap = bp[0:LC, 0:R]
            wf_ap = bp[0:LC, R:2 * R]
            u_ap = {c: bp[0:R, 64 + USLOT[c] * LC:64 + (USLOT[c] + 1) * LC]
                    for c in USLOT}
            nm_ap = bp[0:LC, 184:204]
            id_ap = bp[0:LC, 204:224]
            escn, ebin = fp[:, 0:1], fp[:, 1:2]
            escr, ebir = fp[:, 2:3], fp[:, 3:4]
            onesr = fp[0:1, 4:24]
            zcon = fp[0:1, 24:32]
            zscale = fp[0:1, 32:40]

            def junk_mms(n):
                """PE keep-warm matmuls into a recycled emission psum slot
                (no readers, so they fire whenever the PE is free)."""
                jps = ps_emp.tile([64, 1024], F32, tag="em", name="junk")
                for _ in range(n):
                    nc.tensor.matmul(jps[:, 0:256], junk_w[:], junk_h[:],
                                     start=True, stop=True, perf_mode=DR)

            junk_mms(8)

            # ---- persistent buffers ----
            sw_sb = swp.tile([LC, NCOLI], BF16)
            # z slots: 0 t0_6a, 1 t0_6b, 2 t0_3, 3 sw0_6a, 4 sw0_6b,
            # 5 sw0_3 (DVE free-dim reduces, spread into idle windows)
            zparts = swp.tile([1, 6 * TPC], F32)
            zfin = swp.tile([1, TPC], F32)
            tbs = {}

            def zred(slot, src_ap):
                nc.vector.tensor_reduce(
                    zparts[0:1, slot * TPC:(slot + 1) * TPC],
                    src_ap.rearrange("p (q t) -> p t q", t=TPC), AXX, ADD)
            # y[d]: level-d values in bit-reversed node order (left children
            # of level d-1 in the first half, right in the second)
            y = {}
            for d in range(DEPTH, 0, -1):
                y[d] = ybp.tile([LC, TPC * (1 << d)], BF16,
                                tag=f"y{d}", name=f"y{d}")

            # ---- emission ----
            # tile k -> (chunk, col offset inside chunk)
            TILE_CHUNK = [(0, 0), (0, 512), (1, 0), (1, 512), (1, 1024),
                          (1, 1536), (2, 0), (2, 512), (2, 1024), (2, 1536),
                          (3, 0), (3, 512), (3, 1024), (3, 1536),
                          (4, 0), (4, 512)]

            def em_pair(p):
                """Matmuls for tiles (2p, 2p+1) into one [64,1024] psum."""
                ps = ps_emp.tile([64, 1024], F32, tag="em", name=f"em{p}")
                for j in (0, 1):
                    k = 2 * p + j
                    ci, off = TILE_CHUNK[k]
                    for P in (0, 1):
                        nc.tensor.matmul(
                            ps[:, j * 512:(j + 1) * 512],
                            w_sb[:, P, :, :],
                            htts[ci][:, P, :, off:off + 512],
                            start=(P == 0), stop=(P == 1), perf_mode=DR)
                return ps

            def act_exp_raw(dst, ps, c0, w):
                nc.scalar.activation(dst, ps[32:32 + LC, c0:c0 + w], EXP,
                                     bias=ebir, scale=escr)

            def act_ident_norm(dst, ps, c0, w):
                nc.scalar.activation(dst, ps[0:LC, c0:c0 + w], IDENT,
                                     bias=ebin, scale=escn)

            # ---- tail levels ----
            def level_tile(d, c0, nt):
                """One (c0, nt) tile of internal level d."""
                u = u_ap[C_D[d]]
                H = TPC * (1 << d)        # right-children offset in y[d+1]
                psa = ps_ap.tile([R, 1024], F32, tag="pa", name=f"pa{d}")
                nc.tensor.matmul(psa[:, 0:nt], vf_ap,
                                 y[d + 1][:, c0:c0 + nt],
                                 start=True, stop=True)
                nc.tensor.matmul(psa[:, 512:512 + nt], wf_ap,
                                 y[d + 1][:, H + c0:H + c0 + nt],
                                 start=True, stop=True)
                # DVE cannot read two PSUM inputs: stage W through SBUF
                # (same-engine copy, no cross-engine hop)
                wb = vtp.tile([R, 512], BF16, tag="wb", name=f"wb{d}")
                nc.vector.tensor_copy(wb[:, 0:nt], psa[:, 512:512 + nt])
                vb = vtp.tile([R, 512], BF16, tag="vb", name=f"vb{d}")
                nc.vector.tensor_tensor(vb[:, 0:nt], psa[:, 0:nt],
                                        wb[:, 0:nt], MULT)
                psb = ps_ap.tile([LC, 512], F32, tag="pa", name=f"pb{d}")
                nc.tensor.matmul(psb[:, 0:nt], u, vb[:, 0:nt],
                                 start=True, stop=True)
                if d in FULL:
                    tb = ttp.tile([LC, 512], BF16, tag="tb", name=f"tb{d}")
                    nc.scalar.activation(tb[:, 0:nt], psb[:, 0:nt], LN)
                    tbs[d] = tb
                    psn = ps_ap.tile([LC, 512], F32, tag="pa",
                                     name=f"pn{d}")
                    nc.tensor.matmul(psn[:, 0:nt], nm_ap, tb[:, 0:nt],
                                     start=True, stop=False)
                    nc.tensor.matmul(psn[:, 0:nt], id_ap,
                                     sw_sb[:, OFF[d] + c0:OFF[d] + c0 + nt],
                                     start=False, stop=True)
                    nc.scalar.activation(y[d][:, c0:c0 + nt],
                                         psn[0:LC, 0:nt], EXP)
                else:
                    nc.vector.tensor_tensor(
                        y[d][:, c0:c0 + nt], psb[0:LC, 0:nt],
                        sw_sb[:, OFF[d] + c0:OFF[d] + c0 + nt], MULT)

            # ---- program order ----
            # All emission matmuls for a level-8 tile's inputs retire at
            # least one pair before the tile's own matmuls enter the PE
            # FIFO (strict FIFO: a waiting tail matmul would block later
            # emission matmuls behind it).
            ps = em_pair(0)
            act_exp_raw(y[9][:, 0:512], ps, 0, 512)
            act_exp_raw(y[9][:, 2048:2560], ps, 512, 512)
            ps = em_pair(1)
            act_exp_raw(sw_sb[:, 0:1024], ps, 0, 1024)
            ps = em_pair(2)
            act_exp_raw(sw_sb[:, 1024:2048], ps, 0, 1024)
            level_tile(8, 0, 512)
            ps = em_pair(3)
            act_exp_raw(y[9][:, 512:1024], ps, 0, 512)
            act_exp_raw(y[9][:, 2560:3072], ps, 512, 512)
            ps = em_pair(4)
            act_exp_raw(y[9][:, 1024:1536], ps, 0, 512)
            act_exp_raw(y[9][:, 3072:3584], ps, 512, 512)
            level_tile(8, 512, 512)
            ps = em_pair(5)
            act_exp_raw(y[9][:, 1536:2048], ps, 0, 512)
            act_exp_raw(y[9][:, 3584:4096], ps, 512, 512)
            level_tile(8, 1024, 512)
            ps = em_pair(6)
            act_exp_raw(sw_sb[:, 2048:3072], ps, 0, 1024)
            level_tile(8, 1536, 512)
            # pair 7: L6 sw (norm) + L5..L0
            ps7 = em_pair(7)
            act_ident_norm(sw_sb[:, 3072:3584], ps7, 0, 512)
            act_exp_raw(sw_sb[:, 3584:3968], ps7, 512, 384)
            act_ident_norm(sw_sb[:, 3968:4032], ps7, 896, 64)
            act_exp_raw(sw_sb[:, 4032:4080], ps7, 960, 48)
            nc.scalar.activation(sw_sb[:, 4080:4088], ps7[32:32 + LC,
                                                          1008:1016],
                                 IDENT, bias=ebir, scale=escr)
            level_tile(7, 0, 512)
            zred(3, ps7[0:1, 0:256])        # sw0_6a
            level_tile(7, 512, 512)
            zred(4, ps7[0:1, 256:512])      # sw0_6b
            level_tile(6, 0, 512)
            zred(5, ps7[0:1, 896:960])      # sw0_3 (frees the em slots)
            zred(0, tbs[6][0:1, 0:256])     # t0_6a
            level_tile(5, 0, 256)
            junk_mms(3)
            zred(1, tbs[6][0:1, 256:512])   # t0_6b
            level_tile(4, 0, 128)
            junk_mms(3)
            level_tile(3, 0, 64)
            junk_mms(3)
            zred(2, tbs[3][0:1, 0:64])      # t0_3
            level_tile(2, 0, 32)
            junk_mms(3)
            level_tile(1, 0, 16)
            junk_mms(3)

            # ---- root ----
            psa = ps_ap.tile([R, 1024], F32, tag="pa", name="paR")
            nc.tensor.matmul(psa[:, 0:TPC], vf_ap, y[1][:, 0:TPC],
                             start=True, stop=True)
            nc.tensor.matmul(psa[:, 512:512 + TPC], wf_ap,
                             y[1][:, TPC:2 * TPC],
                             start=True, stop=True)
            wbr = vtp.tile([R, 512], BF16, tag="wb", name="wbR")
            nc.vector.tensor_copy(wbr[:, 0:TPC], psa[:, 512:512 + TPC])
            vb = vtp.tile([R, 512], BF16, tag="vb", name="vbR")
            nc.vector.tensor_tensor(vb[:, 0:TPC], psa[:, 0:TPC],
                                    wbr[:, 0:TPC], MULT)
            psb = ps_ap.tile([LC, 512], F32, tag="pa", name="pbR")
            nc.tensor.matmul(psb[:, 0:TPC], u_ap[12], vb[:, 0:TPC],
                             start=True, stop=True)
            troot = swp.tile([LC, TPC], F32)
            nc.scalar.activation(troot[:], psb[0:LC, 0:TPC], LN)

            # z assembly (zparts: t0 slots 0-2, sw0 slots 3-5)
            zA = swp.tile([1, TPC], F32)
            nc.vector.tensor_reduce(
                zA[:], zparts[0:1, 0:3 * TPC].rearrange(
                    "p (q t) -> p t q", t=TPC), AXX, ADD)
            zB = swp.tile([1, TPC], F32)
            nc.vector.tensor_reduce(
                zB[:], zparts[0:1, 3 * TPC:6 * TPC].rearrange(
                    "p (q t) -> p t q", t=TPC), AXX, ADD)
            z1 = swp.tile([1, TPC], F32)
            nc.gpsimd.tensor_tensor(z1[:], zB[:], zscale, MULT)
            z2 = swp.tile([1, TPC], F32)
            nc.gpsimd.tensor_tensor(z2[:], z1[:], zA[:], ADD)
            nc.gpsimd.tensor_tensor(zfin[:], z2[:], zcon, ADD)

            qps = ps_ap.tile([LC, 512], F32, tag="pa", name="qps")
            nc.tensor.matmul(qps[:, 0:TPC], onesr, zfin[:],
                             start=True, stop=True)
            o1 = swp.tile([LC, TPC], F32)
            nc.vector.tensor_tensor(o1[:], troot[:],
                                    sw_sb[:, 4080:4088], ADD)
            o2 = swp.tile([LC, TPC], F32)
            nc.vector.tensor_tensor(o2[:], o1[:], qps[0:LC, 0:TPC], ADD)
            nc.sync.dma_start(out_d[:], o2[:])

    nc.compile()
    _patch_act_tables(nc)
    return nc


_CACHE = {}


def _get_nc():
    if "nc" not in _CACHE:
        _CACHE["nc"] = _build_bass()
    return _CACHE["nc"]


def run(h, W, b, trans, trace=False, **trace_kwargs):
    h = np.asarray(h, dtype=np.float32)
    W = np.asarray(W, dtype=np.float32)
    b = np.asarray(b, dtype=np.float32)
    trans = np.asarray(trans, dtype=np.float32)

    consts = _host_constants(W, b, trans)
    in_maps = []
    for core in range(NCORES):
        m = dict(consts)
        for i, c in enumerate(_host_ht(h, core)):
            m[f"ht{i}"] = c
        in_maps.append(m)

    nc = _get_nc()
    res = run_bass_kernel_spmd(nc, in_maps, list(range(NCORES)),
                               trace=trace, **trace_kwargs)
    outs = [res.results[k]["out"] for k in range(NCORES)]  # each [20, 8]
    full = np.concatenate([np.asarray(o, np.float32).T for o in outs],
                          axis=0).reshape(B, L, C)
    return np.ascontiguousarray(full), res


def kernel(h, W, b, trans):
    out, _ = run(h, W, b, trans, trace=False)
    return out


# revision 32
# speedup vs baseline: 1.1827x; 1.1827x over previous
"""Trainium2 Bass kernel for BinaryTreeLatentVariable inside algorithm.

Math (per level, bottom-up over a complete binary tree in heap order):
    new[pp, n] = p[pp, n] + logsumexp_{i,j}( trans[pp,i,j] + l[i,n] + r[j,n] )

CP factorization: exp(trans)[pp,i,j] ~= sum_r U[pp,r] V[i,r] W[j,r] (rank-32
ALS fit):
    S[pp, n] = sum_r U[pp,r] * (V^T Fl)[r,n] * (W^T Fr)[r,n]
with F the child values in EXP space.

Level scheme: FULL (ln renormalize, capture per-column z) at levels {6, 3}
only; every other internal level is FAST (pure exp space) with a per-level
power-of-two scale 2^-c_d folded into that level's U matrix to bound the
magnitude drift (values stay within e^+-45; bf16 max e^88).  Host repays
sum(c_d * 2^d) * ln2 + 72*b[0] per tree as a constant.

Per FAST level: mm V^T Fl and W^T Fr into adjacent psum columns (same
partitions 0..31) -> one DVE mult psum*psum -> bf16 vb -> mm U ->
DVE mult by exp(sw_raw) into the level buffer.  No cast, no memsets.
Per FULL level: ... -> mm U -> ACT ln -> psum (normmat@t + I@sw_norm) via
two accumulating matmuls -> ACT exp straight from psum into the level
buffer; t row 0 reduced per tree on GpSimd for z.

Left/right child values live in separate tiles (yL/yR), both at partitions
0..19, parent-column indexed (bit-reversed node order, 8 trees innermost).

Emission: h cast host-side to fp8e4, two DoubleRow matmuls (K=256 each) per
512-col tile into [64,1024] psum super-tiles (two tiles per ACT where the
destination allows), producing a raw block (rows 32..51, consumed via Exp)
and a normalized block (rows 0..19, Identity, for FULL levels) per column.

Column order puts the first leaf L/R pair and the whole L8 block early so
tree levels start while h still streams; 12 junk matmuls at t=0 trip the
PE HAM clock gate to 2.4 GHz before the first real matmul.

Sharding: 8 trees per core across 8 cores (no cross-core communication).
"""

import ml_dtypes
import numpy as np

import concourse.bacc as bacc
import concourse.bass as bass
from concourse import mybir, tile
from concourse.bass_utils import run_bass_kernel_spmd

F32 = mybir.dt.float32
BF16 = mybir.dt.bfloat16
FP8 = mybir.dt.float8e4
NP_BF16 = ml_dtypes.bfloat16
NP_FP8 = ml_dtypes.float8_e4m3

B = 64
N_NODES = 1023
D = 512
L = 5
C = 4
LC = L * C          # 20
NCORES = 8
TPC = B // NCORES   # trees per core = 8
DEPTH = 9           # leaves are level 9; internal levels 8..0
R = 32              # CP rank
LN2 = float(np.log(2.0))

FULL = {6, 3}
# per-level scale exponent: U_d = U * 2^-C_D[d] (host-repaid).  FAST levels
# manage bf16 range; FULL/root values CENTER the ln input near e^0..e^14 --
# the ACT spline is inaccurate far outside that (the scale cancels in t-t0,
# so FULL-level c only moves the ln input and the z constant).
C_D = {8: 20, 7: 16, 6: -50, 5: 20, 4: 16, 3: -52, 2: 20, 1: 12, 0: -43}
ZCON_SHIFT = LN2 * sum(C_D[d] * (1 << d) for d in C_D)  # 10204*ln2
N_FULL_NODES = 64 + 8   # level-6 + level-3 nodes per tree

# sw_sb internal-level offsets (level-major, L8 first)
OFF = {}
_o = 0
for _d in range(8, -1, -1):
    OFF[_d] = _o
    _o += TPC * (1 << _d)
NCOLI = 4096            # internal cols incl 8 pad

# global emission column order (16 tiles of 512):
#  t0 leafL0 | t1 leafR0 | t2-5 L8 | t6 leafL1 | t7 leafR1 | t8 leafL2
#  t9 leafR2 | t10 leafL3 | t11 leafR3 | t12-13 L7 | t14 L6 | t15 rest
CHUNK_COLS = [1024] * 8                  # one 512KB h DMA per emission pair
CHUNK_OFF = [1024 * i for i in range(8)]


def _cp_fit(trans):
    """Rank-R ALS CP fit of exp(trans) as [pp,(lL,lc),(rL,rc)]."""
    T = np.exp(trans.astype(np.float64).transpose(0, 3, 1, 4, 2, 5)
               .reshape(LC, LC, LC))
    rng = np.random.default_rng(0)
    U = rng.uniform(0.5, 1.5, (LC, R))
    V = rng.uniform(0.5, 1.5, (LC, R))
    W = rng.uniform(0.5, 1.5, (LC, R))
    T1 = T.reshape(LC, -1)
    T2 = T.transpose(1, 0, 2).reshape(LC, -1)
    T3 = T.transpose(2, 0, 1).reshape(LC, -1)

    def khatri(A, Bm):
        return (A[:, None, :] * Bm[None, :, :]).reshape(-1, A.shape[1])

    eye = 1e-10 * np.eye(R)
    for _ in range(200):
        for mode in range(3):
            if mode == 0:
                K, M = khatri(V, W), T1
            elif mode == 1:
                K, M = khatri(U, W), T2
            else:
                K, M = khatri(U, V), T3
            X = np.linalg.solve(K.T @ K + eye, (M @ K).T).T
            if mode == 0:
                U = X
            elif mode == 1:
                V = X
            else:
                W = X
    sv = np.abs(V).max(0)
    sw = np.abs(W).max(0)
    return U * (sv * sw), V / sv, W / sw


def _host_constants(W, b, trans):
    W = W.astype(np.float64)
    b = b.astype(np.float64)
    U, Vf, Wf = _cp_fit(trans)

    # emission weights, 64 columns: 0..19 normalized (col i = W_i - W_0,
    # col 0 = W_0 whose psum row is the sw0 z-capture), 32..51 raw
    Wn = np.zeros((D, 64))
    Wn[:, 0] = W[:, 0]
    Wn[:, 1:LC] = W[:, 1:] - W[:, 0:1]
    Wn[:, 32:32 + LC] = W
    esc = float(2.0 ** np.floor(np.log2(235.0 / np.abs(Wn).max())))
    wq = np.clip(Wn * esc, -240, 240).astype(NP_FP8)
    # [p, P, ko, m]: row (P*256 + ko*128 + p) -> w5[p, P, ko, m]
    w5 = np.ascontiguousarray(
        wq.reshape(2, 2, 128, 64).transpose(2, 0, 1, 3))

    # bf16 pack [52, 224]: Vf | Wf | u variants | normmat | idmat
    bpack = np.zeros((52, 224), NP_BF16)
    bpack[0:LC, 0:R] = Vf
    bpack[0:LC, R:2 * R] = Wf
    for j, c in enumerate(sorted(set(C_D.values()))):
        bpack[0:R, 64 + j * LC:64 + (j + 1) * LC] = (U * 2.0 ** (-c)).T
    nm = np.zeros((LC, LC))
    for i in range(1, LC):
        nm[i, i] = 1.0
        nm[0, i] = -1.0
    bpack[0:LC, 184:204] = nm
    bpack[0:LC, 204:224] = np.eye(LC)

    # f32 pack [20, 40]: col0 escn col1 ebin col2 escr col3 ebir;
    # row0: onesr [4:24], zcon [24:32], zscale [32:40]
    fpack = np.zeros((LC, 40), np.float32)
    fpack[1:, 0] = 1.0 / esc
    fpack[1:, 1] = b[1:] - b[0]
    fpack[:, 2] = 1.0 / esc
    fpack[:, 3] = b
    fpack[0, 4:24] = 1.0
    fpack[0, 24:32] = ZCON_SHIFT + N_FULL_NODES * b[0]
    fpack[0, 32:40] = 1.0 / esc
    return {"w5": w5, "bpack": bpack, "fpack": fpack}


def _bitrev(d):
    n = 1 << d
    perm = np.zeros(n, np.int64)
    for x in range(n):
        v, q = x, 0
        for _ in range(d):
            q = (q << 1) | (v & 1)
            v >>= 1
        perm[x] = q
    return perm


def _host_ht(h, core):
    """fp8 chunk list for one core in the new column order."""
    hk = h[core * TPC:(core + 1) * TPC]          # [8, 1023, 512]

    def lvl(d):
        blk = hk[:, (1 << d) - 1:(1 << (d + 1)) - 1, :]
        blk = blk[:, _bitrev(d), :]
        return blk.transpose(2, 1, 0).reshape(D, -1)   # col = x*8+t

    l9 = lvl(9)                 # [512, 4096]
    l9L, l9R = l9[:, :2048], l9[:, 2048:]
    blocks = [l9L[:, 0:512], l9R[:, 0:512], lvl(8),
              l9L[:, 512:1024], l9R[:, 512:1024],
              l9L[:, 1024:1536], l9R[:, 1024:1536],
              l9L[:, 1536:2048], l9R[:, 1536:2048],
              lvl(7), lvl(6), lvl(5), lvl(4), lvl(3), lvl(2), lvl(1),
              lvl(0), np.zeros((D, 8), np.float32)]
    out = np.concatenate(blocks, axis=1)          # [512, 8192]
    hq = np.clip(out, -240, 240).astype(NP_FP8)
    # row (P*256 + ko*128 + p) -> [p, P, ko, col]
    h4 = hq.reshape(2, 2, 128, 8192).transpose(2, 0, 1, 3)
    return [np.ascontiguousarray(h4[:, :, :, o:o + n])
            for o, n in zip(CHUNK_OFF, CHUNK_COLS)]


def _patch_act_tables(nc):
    """Retarget every activation-table load to natural_log_exp_and_others
    (covers Exp, Ln and Identity) and drop the now-redundant reloads."""
    from concourse.hw_specs import get_activation_tables
    tables = list(get_activation_tables(nc.m.arch).items())
    target = None
    for idx, (name, _fns) in enumerate(tables):
        if name == "natural_log_exp_and_others":
            target = idx
    if target is None:
        return
    for fn in nc.m.functions:
        kept = False
        for blk in fn.blocks:
            new_insts = []
            for ins in blk.instructions:
                if isinstance(ins, mybir.InstLoadActFuncSet):
                    si = ins.sync_info
                    has_sems = si is not None and (
                        len(si.on_wait) > 0 or len(si.on_update) > 0)
                    if not kept or has_sems:
                        ins.act_func_set_id = target
                        kept = True
                        new_insts.append(ins)
                    continue
                new_insts.append(ins)
            blk.instructions[:] = new_insts


def _build_bass():
    nc = bacc.Bacc("TRN2", target_bir_lowering=False)

    ht_d = [nc.declare_dram_parameter(f"ht{i}", [128, 2, 2, n], FP8,
                                      isOutput=False)
            for i, n in enumerate(CHUNK_COLS)]
    w5_d = nc.declare_dram_parameter("w5", [128, 2, 2, 64], FP8,
                                     isOutput=False)
    bpack_d = nc.declare_dram_parameter("bpack", [52, 224], BF16,
                                        isOutput=False)
    fpack_d = nc.declare_dram_parameter("fpack", [LC, 40], F32,
                                        isOutput=False)
    out_d = nc.declare_dram_parameter("out", [LC, TPC], F32, isOutput=True)

    EXP = mybir.ActivationFunctionType.Exp
    LN = mybir.ActivationFunctionType.Ln
    IDENT = mybir.ActivationFunctionType.Identity
    ADD = mybir.AluOpType.add
    MULT = mybir.AluOpType.mult
    DR = mybir.MatmulPerfMode.DoubleRow
    AXX = mybir.AxisListType.X

    USLOT = {c: j for j, c in enumerate(sorted(set(C_D.values())))}
    # zparts slots: 0 t0_L6, 1 t0_L3, 2 sw0_L6, 3 sw0_L3
    with tile.TileContext(nc) as tc:
        with (
            tc.tile_pool(name="consts", bufs=1) as consts,
            tc.tile_pool(name="sw", bufs=1) as swp,
            tc.tile_pool(name="ybufs", bufs=1) as ybp,
            tc.tile_pool(name="ht", bufs=2) as htp,
            tc.tile_pool(name="vtiles", bufs=2) as vtp,
            tc.tile_pool(name="ttiles", bufs=2) as ttp,
            tc.tile_pool(name="ps_em", bufs=2, space="PSUM") as ps_emp,
            tc.tile_pool(name="ps_a", bufs=2, space="PSUM") as ps_ap,
        ):
            # ---- DMAs first: h chunks on sync queue, consts on scalar ----
            # bufs=1 per tag: at most 2 transfers in flight, so the
            # SDMA packet round-robin can't starve the next-needed chunk
            htts = []
            for i, ncol in enumerate(CHUNK_COLS):
                t = htp.tile([128, 2, 2, ncol], FP8, tag=f"ht{i % 2}",
                             name=f"ht{i}", bufs=1)
                nc.sync.dma_start(t[:], ht_d[i][:])
                htts.append(t)
            # junk tiles for PE warm-up matmuls (memsets first so the
            # junk matmuls start immediately)
            junk_w = consts.tile([128, 2, 64], FP8)
            junk_h = consts.tile([128, 2, 256], FP8)
            nc.gpsimd.memset(junk_w[:], 0.0)
            nc.gpsimd.memset(junk_h[:], 0.0)
            # consts via gpsimd SWDGE: keeps the scalar/ACT queue free so
            # the activation-table load runs at t=0
            w_sb = consts.tile([128, 2, 2, 64], FP8)
            nc.gpsimd.dma_start(w_sb[:], w5_d[:])
            bp = consts.tile([52, 224], BF16)
            nc.gpsimd.dma_start(bp[:], bpack_d[:])
            fp = consts.tile([LC, 40], F32)
            nc.gpsimd.dma_start(fp[:], fpack_d[:])

            vf_ap = bp[0:LC, 0:R]
            wf_ap = bp[0:LC, R:2 * R]
            u_ap = {c: bp[0:R, 64 + USLOT[c] * LC:64 + (USLOT[c] + 1) * LC]
                    for c in USLOT}
            nm_ap = bp[0:LC, 184:204]
            id_ap = bp[0:LC, 204:224]
            escn, ebin = fp[:, 0:1], fp[:, 1:2]
            escr, ebir = fp[:, 2:3], fp[:, 3:4]
            onesr = fp[0:1, 4:24]
            zcon = fp[0:1, 24:32]
            zscale = fp[0:1, 32:40]

            def junk_mms(n):
                """PE keep-warm matmuls into a recycled emission psum slot
                (no readers, so they fire whenever the PE is free)."""
                jps = ps_emp.tile([64, 1024], F32, tag="em", name="junk")
                for _ in range(n):
                    nc.tensor.matmul(jps[:, 0:256], junk_w[:], junk_h[:],
                                     start=True, stop=True, perf_mode=DR)

            # dummy activation so walrus places the ACT table load at t=0
            # (contents irrelevant, tile never read)
            dummy_act = consts.tile([1, 16], F32)
            nc.scalar.activation(dummy_act[0:1, 0:8], dummy_act[0:1, 8:16],
                                 IDENT)
            junk_mms(8)

            # ---- persistent buffers ----
            sw_sb = swp.tile([LC, NCOLI], BF16)
            # z slots: 0 t0_6a, 1 t0_6b, 2 t0_3, 3 sw0_6a, 4 sw0_6b,
            # 5 sw0_3 (DVE free-dim reduces, spread into idle windows)
            zparts = swp.tile([1, 6 * TPC], F32)
            zfin = swp.tile([1, TPC], F32)
            tbs = {}

            def zred(slot, src_ap):
                nc.vector.tensor_reduce(
                    zparts[0:1, slot * TPC:(slot + 1) * TPC],
                    src_ap.rearrange("p (q t) -> p t q", t=TPC), AXX, ADD)
            # y[d]: level-d values in bit-reversed node order (left children
            # of level d-1 in the first half, right in the second)
            y = {}
            for d in range(DEPTH, 0, -1):
                y[d] = ybp.tile([LC, TPC * (1 << d)], BF16,
                                tag=f"y{d}", name=f"y{d}")

            # ---- emission ----
            # tile k -> (chunk, col offset inside chunk)
            TILE_CHUNK = [(k // 2, (k % 2) * 512) for k in range(16)]

            def em_pair(p):
                """Matmuls for tiles (2p, 2p+1) into one [64,1024] psum."""
                ps = ps_emp.tile([64, 1024], F32, tag="em", name=f"em{p}")
                for j in (0, 1):
                    k = 2 * p + j
                    ci, off = TILE_CHUNK[k]
                    for P in (0, 1):
                        nc.tensor.matmul(
                            ps[:, j * 512:(j + 1) * 512],
                            w_sb[:, P, :, :],
                            htts[ci][:, P, :, off:off + 512],
                            start=(P == 0), stop=(P == 1), perf_mode=DR)
                return ps

            def act_exp_raw(dst, ps, c0, w):
                nc.scalar.activation(dst, ps[32:32 + LC, c0:c0 + w], EXP,
                                     bias=ebir, scale=escr)

            def act_ident_norm(dst, ps, c0, w):
                nc.scalar.activation(dst, ps[0:LC, c0:c0 + w], IDENT,
                                     bias=ebin, scale=escn)

            # ---- tail levels ----
            def level_tile(d, c0, nt):
                """One (c0, nt) tile of internal level d."""
                u = u_ap[C_D[d]]
                H = TPC * (1 << d)        # right-children offset in y[d+1]
                psa = ps_ap.tile([R, 1024], F32, tag="pa", name=f"pa{d}")
                nc.tensor.matmul(psa[:, 0:nt], vf_ap,
                                 y[d + 1][:, c0:c0 + nt],
                                 start=True, stop=True)
                nc.tensor.matmul(psa[:, 512:512 + nt], wf_ap,
                                 y[d + 1][:, H + c0:H + c0 + nt],
                                 start=True, stop=True)
                # DVE cannot read two PSUM inputs: stage W through SBUF
                # (same-engine copy, no cross-engine hop)
                wb = vtp.tile([R, 512], BF16, tag="wb", name=f"wb{d}")
                nc.vector.tensor_copy(wb[:, 0:nt], psa[:, 512:512 + nt])
                vb = vtp.tile([R, 512], BF16, tag="vb", name=f"vb{d}")
                nc.vector.tensor_tensor(vb[:, 0:nt], psa[:, 0:nt],
                                        wb[:, 0:nt], MULT)
                psb = ps_ap.tile([LC, 512], F32, tag="pa", name=f"pb{d}")
                nc.tensor.matmul(psb[:, 0:nt], u, vb[:, 0:nt],
                                 start=True, stop=True)
                if d in FULL:
                    tb = ttp.tile([LC, 512], BF16, tag="tb", name=f"tb{d}")
                    nc.scalar.activation(tb[:, 0:nt], psb[:, 0:nt], LN)
                    tbs[d] = tb
                    psn = ps_ap.tile([LC, 512], F32, tag="pa",
                                     name=f"pn{d}")
                    nc.tensor.matmul(psn[:, 0:nt], nm_ap, tb[:, 0:nt],
                                     start=True, stop=False)
                    nc.tensor.matmul(psn[:, 0:nt], id_ap,
                                     sw_sb[:, OFF[d] + c0:OFF[d] + c0 + nt],
                                     start=False, stop=True)
                    nc.scalar.activation(y[d][:, c0:c0 + nt],
                                         psn[0:LC, 0:nt], EXP)
                else:
                    nc.vector.tensor_tensor(
                        y[d][:, c0:c0 + nt], psb[0:LC, 0:nt],
                        sw_sb[:, OFF[d] + c0:OFF[d] + c0 + nt], MULT)

            # ---- program order ----
            # All emission matmuls for a level-8 tile's inputs retire at
            # least one pair before the tile's own matmuls enter the PE
            # FIFO (strict FIFO: a waiting tail matmul would block later
            # emission matmuls behind it).
            ps = em_pair(0)
            act_exp_raw(y[9][:, 0:512], ps, 0, 512)
            act_exp_raw(y[9][:, 2048:2560], ps, 512, 512)
            ps = em_pair(1)
            act_exp_raw(sw_sb[:, 0:1024], ps, 0, 1024)
            junk_mms(2)
            ps = em_pair(2)
            act_exp_raw(sw_sb[:, 1024:2048], ps, 0, 1024)
            level_tile(8, 0, 512)
            ps = em_pair(3)
            act_exp_raw(y[9][:, 512:1024], ps, 0, 512)
            act_exp_raw(y[9][:, 2560:3072], ps, 512, 512)
            junk_mms(2)
            ps = em_pair(4)
            act_exp_raw(y[9][:, 1024:1536], ps, 0, 512)
            act_exp_raw(y[9][:, 3072:3584], ps, 512, 512)
            level_tile(8, 512, 512)
            ps = em_pair(5)
            act_exp_raw(y[9][:, 1536:2048], ps, 0, 512)
            act_exp_raw(y[9][:, 3584:4096], ps, 512, 512)
            junk_mms(2)
            level_tile(8, 1024, 512)
            ps = em_pair(6)
            act_exp_raw(sw_sb[:, 2048:3072], ps, 0, 1024)
            level_tile(8, 1536, 512)
            # pair 7: L6 sw (norm) + L5..L0
            ps7 = em_pair(7)
            act_ident_norm(sw_sb[:, 3072:3584], ps7, 0, 512)
            act_exp_raw(sw_sb[:, 3584:3968], ps7, 512, 384)
            act_ident_norm(sw_sb[:, 3968:4032], ps7, 896, 64)
            act_exp_raw(sw_sb[:, 4032:4080], ps7, 960, 48)
            nc.scalar.activation(sw_sb[:, 4080:4088], ps7[32:32 + LC,
                                                          1008:1016],
                                 IDENT, bias=ebir, scale=escr)
            level_tile(7, 0, 512)
            zred(3, ps7[0:1, 0:256])        # sw0_6a
            level_tile(7, 512, 512)
            zred(4, ps7[0:1, 256:512])      # sw0_6b
            level_tile(6, 0, 512)
            zred(5, ps7[0:1, 896:960])      # sw0_3 (frees the em slots)
            zred(0, tbs[6][0:1, 0:256])     # t0_6a
            level_tile(5, 0, 256)
            junk_mms(3)
            zred(1, tbs[6][0:1, 256:512])   # t0_6b
            level_tile(4, 0, 128)
            junk_mms(3)
            level_tile(3, 0, 64)
            junk_mms(3)
            zred(2, tbs[3][0:1, 0:64])      # t0_3
            level_tile(2, 0, 32)
            junk_mms(3)
            level_tile(1, 0, 16)
            junk_mms(3)

            # ---- root ----
            psa = ps_ap.tile([R, 1024], F32, tag="pa", name="paR")
            nc.tensor.matmul(psa[:, 0:TPC], vf_ap, y[1][:, 0:TPC],
                             start=True, stop=True)
            nc.tensor.matmul(psa[:, 512:512 + TPC], wf_ap,
                             y[1][:, TPC:2 * TPC],
                             start=True, stop=True)
            wbr = vtp.tile([R, 512], BF16, tag="wb", name="wbR")
            nc.vector.tensor_copy(wbr[:, 0:TPC], psa[:, 512:512 + TPC])
            vb = vtp.tile([R, 512], BF16, tag="vb", name="vbR")
            nc.vector.tensor_tensor(vb[:, 0:TPC], psa[:, 0:TPC],
                                    wbr[:, 0:TPC], MULT)
            psb = ps_ap.tile([LC, 512], F32, tag="pa", name="pbR")
            nc.tensor.matmul(psb[:, 0:TPC], u_ap[12], vb[:, 0:TPC],
                             start=True, stop=True)
            troot = swp.tile([LC, TPC], F32)
            nc.scalar.activation(troot[:], psb[0:LC, 0:TPC], LN)

            # z assembly (zparts: t0 slots 0-2, sw0 slots 3-5)
            zA = swp.tile([1, TPC], F32)
            nc.vector.tensor_reduce(
                zA[:], zparts[0:1, 0:3 * TPC].rearrange(
                    "p (q t) -> p t q", t=TPC), AXX, ADD)
            zB = swp.tile([1, TPC], F32)
            nc.vector.tensor_reduce(
                zB[:], zparts[0:1, 3 * TPC:6 * TPC].rearrange(
                    "p (q t) -> p t q", t=TPC), AXX, ADD)
            z1 = swp.tile([1, TPC], F32)
            nc.gpsimd.tensor_tensor(z1[:], zB[:], zscale, MULT)
            z2 = swp.tile([1, TPC], F32)
            nc.gpsimd.tensor_tensor(z2[:], z1[:], zA[:], ADD)
            nc.gpsimd.tensor_tensor(zfin[:], z2[:], zcon, ADD)

            qps = ps_ap.tile([LC, 512], F32, tag="pa", name="qps")
            nc.tensor.matmul(qps[:, 0:TPC], onesr, zfin[:],
                             start=True, stop=True)
            o1 = swp.tile([LC, TPC], F32)
            nc.vector.tensor_tensor(o1[:], troot[:],
                                    sw_sb[:, 4080:4088], ADD)
            o2 = swp.tile([LC, TPC], F32)
            nc.vector.tensor_tensor(o2[:], o1[:], qps[0:LC, 0:TPC], ADD)
            nc.sync.dma_start(out_d[:], o2[:])

    nc.compile()
    _patch_act_tables(nc)
    return nc


_CACHE = {}


def _get_nc():
    if "nc" not in _CACHE:
        _CACHE["nc"] = _build_bass()
    return _CACHE["nc"]


def run(h, W, b, trans, trace=False, **trace_kwargs):
    h = np.asarray(h, dtype=np.float32)
    W = np.asarray(W, dtype=np.float32)
    b = np.asarray(b, dtype=np.float32)
    trans = np.asarray(trans, dtype=np.float32)

    consts = _host_constants(W, b, trans)
    in_maps = []
    for core in range(NCORES):
        m = dict(consts)
        for i, c in enumerate(_host_ht(h, core)):
            m[f"ht{i}"] = c
        in_maps.append(m)

    nc = _get_nc()
    res = run_bass_kernel_spmd(nc, in_maps, list(range(NCORES)),
                               trace=trace, **trace_kwargs)
    outs = [res.results[k]["out"] for k in range(NCORES)]  # each [20, 8]
    full = np.concatenate([np.asarray(o, np.float32).T for o in outs],
                          axis=0).reshape(B, L, C)
    return np.ascontiguousarray(full), res


def kernel(h, W, b, trans):
    out, _ = run(h, W, b, trans, trace=False)
    return out
